# revision 14
# baseline (speedup 1.0000x reference)
"""Trainium2 Bass kernel for nn_CCL__69277822485245 (spectral conv via DCT/FFT).

Math: the reference's rFFT along W cancels into a circular 5-tap convolution,
and the DCT-II sandwich M @ diag(D[:,s]) @ D collapses into 5 dense 128x128
matrices G_s (precomputed on host). Per batch element:

    u_s[i, m, w] = sum_h G_s[m, h] x[i, h, w]                  (stage 1)
    out[o, m, n] = sum_{s,t,i} W[o,i,s,t] u_s[i, m, (n-t)%W] + bias[o]   (stage 2)

Sharding: data-parallel over batch B=8 across the 8 NeuronCores (1 each).

Key trick (stage 1): G_s is persymmetric (G_s[127-m,127-h] = G_s[m,h], a DCT
parity fact), so with x split into h-parity parts xe/xo and a sign-flipped
duplicate in the stationary (d=0 columns hold (xe,xo), d=1 hold (xe,-xo)),
one K=128, N=320 matmul per w-column yields BOTH m-halves at once:
psum[0:64] = u_s[i, m<64], psum[64:128] = u_s[i, 127-m]. Stage-1 PE cost
halves vs. the naive duplicated-x formulation.

u tiles (per m-half) carry K=128 (i, s-pair) rows for stage 2: lower
partitions = even s slots {0,2,4}, upper = {1, 3, s4-shifted}. Odd-s data is
staged through a small SBUF scratch ring and hopped across the partition
boundary by SBUF->SBUF DMAs; the s=4 slot additionally gets a one-w-shifted
copy so stage 2 runs 13 K=128 accumulation groups (10 s-pairs x t + 3
(s4,t)&(s4,t+1) pairs). The mh=1 tile has the partition halves swapped
(its weight stack is row-swapped) and its m axis reversed (host fixes up).

Stage-1 psum->SBUF casts alternate DVE / Scalar; output chunks are written
contiguously as [mh*16+ch, o, (j,m)] and reassembled on host.
"""

import numpy as np

H = 128
W = 128
CI = 64
CO = 128
KH = 5
KW = 5
B = 8

MH = 64          # m-half
HALO = 4         # wrap columns at the front of u's j axis
JW = W + HALO    # 132 j-columns; w = (j - 4) % 128
NCH = 16         # stage-2 output chunks per m-half (8 w-columns each)
LA = 4           # stage-1 tile emission lookahead beyond chunk needs
NT = JW // 2     # 66 stage-1 tiles (2 w-columns each)

DTYPE = "bf16"

_PROG = None
_CONSTS = None
_RUN_OPTS = {}     # test harness may set e.g. {"trace": True, "trace_cores": [0]}
_LAST_RESULT = None

G10 = [(t, c) for t in range(KH) for c in range(2)]   # K=128 (i, s=2c / 2c+1)
SORD = [0, 2, 4, 1, 3]


def _np_dt():
    if DTYPE == "bf16":
        import ml_dtypes
        return ml_dtypes.bfloat16
    return np.float32


def _build_consts():
    n = np.arange(H, dtype=np.float64)
    ang = np.pi * (2.0 * n[None, :] + 1.0) * n[:, None] / (2.0 * H)  # [k, h]
    D = 2.0 * np.cos(ang)
    wgt = np.where(n == 0, 0.5, 1.0)
    M = (np.cos(ang).T * wgt[None, :]) / (2.0 * H)                    # [m, k]
    G = [M @ (D[:, s:s + 1] * D) for s in range(KH)]                  # [m, h]
    # rows 0:64 = Ge (h-even part), 64:128 = Go; cols = (sidx, m<64)
    gt = np.zeros((H, KH * MH))
    for q, s in enumerate(SORD):
        gt[0:64, q * MH:(q + 1) * MH] = (G[s][:64, :64].T
                                         + G[s][:64, 127:63:-1].T)
        gt[64:128, q * MH:(q + 1) * MH] = (G[s][:64, :64].T
                                           - G[s][:64, 127:63:-1].T)
    return np.ascontiguousarray(gt).astype(_np_dt())


def _build_program():
    import concourse.mybir as mybir
    import concourse.tile as tile
    from concourse import bacc

    f32 = mybir.dt.float32
    mmdt = {"bf16": mybir.dt.bfloat16,
            "f32r": mybir.dt.float32r,
            "f32": mybir.dt.float32}[DTYPE]

    nc = bacc.Bacc("TRN2", target_bir_lowering=False, debug=False,
                   enable_asserts=False, num_devices=B)
    x_d = nc.dram_tensor("x", [H, W, CI], mmdt, kind="ExternalInput").ap()
    g_d = nc.dram_tensor("g", [H, KH * MH], mmdt, kind="ExternalInput").ap()
    w_d = nc.dram_tensor("wt", [128, 2 * 13 * CO], mmdt,
                         kind="ExternalInput").ap()
    b_d = nc.dram_tensor("bias", [CO, 1], f32, kind="ExternalInput").ap()
    o_d = nc.dram_tensor("out", [2 * NCH, CO, 512], f32,
                         kind="ExternalOutput").ap()

    with tile.TileContext(nc) as tc:
        with (
            tc.tile_pool(name="const", bufs=1) as cpool,
            tc.tile_pool(name="u", bufs=1) as upool,
            tc.tile_pool(name="scr", bufs=3) as spool,
            tc.tile_pool(name="oacc", bufs=3) as opool,
            tc.tile_pool(name="ps1", bufs=3, space="PSUM") as ps1,
            tc.tile_pool(name="ps2", bufs=2, space="PSUM") as ps2,
        ):
            # DMA descriptors drain FIFO per engine; order by first use.
            gt = cpool.tile([H, KH * MH], mmdt)
            nc.sync.dma_start(gt[:], g_d)
            xT = cpool.tile([H, W * 2 * CI], mmdt)
            x4 = xT[:].rearrange("p (w di) -> p w di", di=2 * CI)

            def load_x_chunk(wc):
                # HBM load of (xe; xo) rows, then build the sign-flipped
                # duplicate on-chip: d=1 columns = (xe, -xo).
                sl = slice(wc * 16, (wc + 1) * 16)
                nc.sync.dma_start(x4[:, sl, 0:CI], x_d[:, sl, :])
                nc.vector.tensor_copy(x4[0:64, sl, CI:2 * CI],
                                      x4[0:64, sl, 0:CI])
                nc.vector.tensor_scalar_mul(x4[64:128, sl, CI:2 * CI],
                                            x4[64:128, sl, 0:CI], -1.0)

            for wc in [7, 0]:
                load_x_chunk(wc)
            wt = cpool.tile([128, 2 * 13 * CO], mmdt)
            nc.sync.dma_start(wt[:], w_d)
            bt = cpool.tile([CO, 1], f32)
            nc.sync.dma_start(bt[:], b_d)
            for wc in [1, 2, 3, 4, 5, 6]:
                load_x_chunk(wc)

            import concourse.mybir as _mb

            u0 = upool.tile([128, 3 * JW * MH], mmdt, tag="u0")
            u0v = u0[:].rearrange("p (c j m) -> p c j m", c=3, j=JW)
            u1 = upool.tile([128, 3 * JW * MH], mmdt, tag="u1")
            u1v = u1[:].rearrange("p (c j m) -> p c j m", c=3, j=JW)
            # j=0 of the shifted s4 slots is never valid data; zero it so the
            # (zero-weighted) t=4 pair reads don't hit NaNs.
            nc.vector.memset(u0v[64:128, 2, 0:1, :], 0.0)
            nc.vector.memset(u1v[0:64, 2, 0:1, :], 0.0)

            state = {"scr": None, "scrv": None}

            def s1_tile(jp):
                if jp % 4 == 0:
                    s = spool.tile([128, 2 * 8 * MH], mmdt)
                    state["scr"] = s
                    state["scrv"] = s[:].rearrange("p (c j m) -> p c j m",
                                                   c=2, j=8)
                scv = state["scrv"]
                p1 = ps1.tile([128, 1024], f32)
                for dj in range(2):
                    wg = (2 * jp + dj - HALO) % W
                    nc.tensor.matmul(p1[:, dj * 512:dj * 512 + KH * MH],
                                     x4[:, wg, :], gt[:],
                                     start=True, stop=True)
                pv = p1[:].rearrange("p (j s m) -> p j s m", j=2, s=8)
                jl = (2 * jp) % 8
                # evens (s 0,2,4) -> slots 0:3 of the same partition half
                nc.vector.tensor_copy(
                    u0v[0:64, :, 2 * jp:2 * jp + 2, :].transpose([0, 2, 1, 3]),
                    pv[0:64, :, 0:3, :])
                nc.scalar.copy(
                    u1v[64:128, :, 2 * jp:2 * jp + 2, :].transpose([0, 2, 1, 3]),
                    pv[64:128, :, 0:3, :])
                # odds (s 1,3) -> scratch ring, hopped cross-partition by DMA
                nc.vector.tensor_copy(
                    scv[0:64, :, jl:jl + 2, :].transpose([0, 2, 1, 3]),
                    pv[0:64, :, 3:5, :])
                nc.scalar.copy(
                    scv[64:128, :, jl:jl + 2, :].transpose([0, 2, 1, 3]),
                    pv[64:128, :, 3:5, :])
                if jp % 4 == 3 or jp == NT - 1:
                    p = jp // 4
                    j0, j1 = 8 * p, min(8 * p + 8, JW)
                    jn = j1 - j0
                    nc.sync.dma_start(u0v[64:128, 0:2, j0:j1, :],
                                      scv[0:64, :, 0:jn, :])
                    nc.sync.dma_start(u1v[0:64, 0:2, j0:j1, :],
                                      scv[64:128, :, 0:jn, :])
                    # shifted s4 copies: us4sh[j] = u_s4[j-1]
                    s0, s1_ = max(8 * p, 1), j1
                    nc.sync.dma_start(u0v[64:128, 2, s0:s1_, :],
                                      u0v[0:64, 2, s0 - 1:s1_ - 1, :])
                    nc.sync.dma_start(u1v[0:64, 2, s0:s1_, :],
                                      u1v[64:128, 2, s0 - 1:s1_ - 1, :])

            def s2_chunk(mh, ch):
                uv = u0v if mh == 0 else u1v
                wof = mh * 13 * CO
                p2 = ps2.tile([128, 512], f32)
                for gi, (t, c) in enumerate(G10):
                    rhs = uv[0:128, c, ch * 8 + HALO - t:ch * 8 + HALO - t + 8, :]
                    nc.tensor.matmul(p2[:], wt[:, wof + gi * CO:
                                                wof + (gi + 1) * CO], rhs,
                                     start=(gi == 0), stop=False)
                for p in range(3):
                    tp = 2 * p
                    rhs = uv[0:128, 2,
                             ch * 8 + HALO - tp:ch * 8 + HALO - tp + 8, :]
                    nc.tensor.matmul(p2[:], wt[:, wof + (10 + p) * CO:
                                                wof + (11 + p) * CO],
                                     rhs, start=False, stop=(p == 2))
                oa = opool.tile([CO, 512], f32)
                nc.scalar.activation(oa[:], p2[:],
                                     _mb.ActivationFunctionType.Identity,
                                     bias=bt[:])
                nc.sync.dma_start(o_d[mh * NCH + ch], oa[:])

            ti = 0
            for k in range(NCH):
                target = min(4 * k + 7 + LA, NT)
                while ti < target:
                    s1_tile(ti)
                    ti += 1
                s2_chunk(0, k)
                s2_chunk(1, k)
            while ti < NT:
                s1_tile(ti)
                ti += 1
    nc.compile()
    return nc


def _get_prog():
    global _PROG
    if _PROG is None:
        _PROG = _build_program()
    return _PROG


def _build_wstack(weight):
    # wst0 (mh=0): 13 groups; g<10: rows 0:64 s=2c, 64:128 s=2c+1;
    # g=10+p: rows 0:64 (s=4, t=2p), rows 64:128 (s=4, t=2p+1) (zeros p=2).
    # wst1 (mh=1): partition halves of u are swapped -> row-swapped stack.
    wst0 = np.zeros((128, 13 * CO), np.float32)
    for gi, (t, c) in enumerate(G10):
        col = gi * CO
        wst0[0:64, col:col + CO] = weight[:, :, 2 * c, t].T
        wst0[64:128, col:col + CO] = weight[:, :, 2 * c + 1, t].T
    for p in range(3):
        col = (10 + p) * CO
        tp = 2 * p
        wst0[0:64, col:col + CO] = weight[:, :, 4, tp].T
        if tp + 1 < KW:
            wst0[64:128, col:col + CO] = weight[:, :, 4, tp + 1].T
    wst1 = np.concatenate([wst0[64:128], wst0[0:64]], axis=0)
    wst = np.concatenate([wst0, wst1], axis=1)
    return np.ascontiguousarray(wst).astype(_np_dt())


def kernel(x, weight, bias):
    from concourse.bass_utils import run_bass_kernel_spmd

    global _CONSTS
    if _CONSTS is None:
        _CONSTS = _build_consts()
    GT = _CONSTS

    x = np.ascontiguousarray(np.asarray(x, dtype=np.float32))
    weight = np.ascontiguousarray(np.asarray(weight, dtype=np.float32))
    bias = np.ascontiguousarray(np.asarray(bias, dtype=np.float32))

    wst = _build_wstack(weight)
    b2 = np.ascontiguousarray(bias.reshape(CO, 1))

    in_maps = []
    for b in range(B):
        xh = x[b].transpose(1, 2, 0)                       # [h, w, i]
        xe = (xh[0:64] + xh[127:63:-1]) * 0.5
        xo = (xh[0:64] - xh[127:63:-1]) * 0.5
        xp = np.concatenate([xe, xo], axis=0).reshape(H, W * CI)
        in_maps.append({"x": np.ascontiguousarray(xp).astype(_np_dt()),
                        "g": GT, "wt": wst, "bias": b2})

    res = run_bass_kernel_spmd(_get_prog(), in_maps, core_ids=list(range(B)),
                               **_RUN_OPTS)
    global _LAST_RESULT
    _LAST_RESULT = res
    outs = []
    for b in range(B):
        arr = res.results[b]["out"].reshape(2, NCH, CO, 8, MH)
        out = np.empty((CO, H, W), np.float32)
        out[:, 0:MH, :] = arr[0].transpose(1, 3, 0, 2).reshape(CO, MH, W)
        out[:, MH:H, :] = arr[1][:, :, :, ::-1].transpose(1, 3, 0, 2) \
                                .reshape(CO, MH, W)
        outs.append(out)
    out = np.stack(outs, axis=0)
    return np.ascontiguousarray(out.astype(np.float32))
